# revision 13
# baseline (speedup 1.0000x reference)
"""MoE layer (top-2 of 8 experts) on 8 TRN2 NeuronCores, expert-parallel.

Host side: router (exact replica of the reference jax ops, so top-k
selection bit-matches), token gather by expert assignment, weight
repacking into DMA-friendly layouts + cast to bf16, and the final
weighted scatter-add.

Device side (one expert per core, SPMD): the full expert FFN
    h = X @ W1 ; act = gelu(h_gate) * h_up ; Y = act @ W2
matmuls run in bf16 (full PE rate, fast weight load, ~3e-3 rel err)
accumulating in fp32 PSUM, with all activations kept transposed
(tokens on the free axis) so no on-device transposes are needed.
Token chunks iterate *inside* the k-accumulation loop so one weight
load feeds all chunks and the PE streams wall-to-wall.

Self-contained: only library imports (numpy/jax/ml_dtypes/concourse).
"""

import numpy as np
import ml_dtypes

BF16 = ml_dtypes.bfloat16

TOP_K = 2
EPS = 1e-6
P = 128
D = 2048
F = 2048  # expert hidden dim (ED)
E = 8
KO = D // P  # 16 K-tiles for matmul1 / output D-tiles
MJ = F // P  # 16 gate/up tile pairs; also K-tiles for matmul2

_BUILD_CACHE: dict = {}

# Activation for the gate branch. CoreSim doesn't implement Gelu, so tests
# can set this to "Identity" for structural sim validation.
ACT_FN = "Gelu"


def _chunks_of(C: int) -> list[tuple[int, int]]:
    """Split the token-capacity free axis into equal matmul chunks <= 512."""
    if C <= 512:
        return [(0, C)]
    nch = -(-C // 512)
    base = C // nch
    base -= base % 8
    sizes = [base] * nch
    rem = C - base * nch
    i = 0
    while rem > 0:
        add = min(8, rem)
        sizes[i % nch] += add
        rem -= add
        i += 1
    out = []
    off = 0
    for s in sizes:
        out.append((off, s))
        off += s
    assert off == C
    return out


def _build(C: int):
    """Build + compile the per-core expert-FFN bass program for capacity C."""
    key = (C, ACT_FN)
    if key in _BUILD_CACHE:
        return _BUILD_CACHE[key]

    import concourse.bacc as bacc
    import concourse.mybir as mybir
    import concourse.tile as tile
    f32 = mybir.dt.float32
    bf16 = mybir.dt.bfloat16
    act_fn = getattr(mybir.ActivationFunctionType, ACT_FN)
    chunks = _chunks_of(C)

    nc = bacc.Bacc(
        "TRN2", target_bir_lowering=False, debug=False, enable_asserts=False
    )
    # Packed layouts (host pre-transposed, partition-major, bf16):
    #   xt[p, ko, c]    = X^T[ko*128+p, c]          (tokens on free axis)
    #   w1[p, m, ko, q] = W1perm[ko*128+p, m*128+q] (m: g0,u0,g1,u1,... strips)
    #   w2[p, i, fo, q] = W2[fo*128+p, i*128+q]
    #   yt[p, io, c]    = Y^T[io*128+p, c]
    xt_d = nc.dram_tensor("xt", [P, KO, C], bf16, kind="ExternalInput")
    w1_d = nc.dram_tensor("w1", [P, 2 * MJ, KO, P], bf16, kind="ExternalInput")
    w2_d = nc.dram_tensor("w2", [P, KO, MJ, P], bf16, kind="ExternalInput")
    yt_d = nc.dram_tensor("yt", [P, KO, C], f32, kind="ExternalOutput")

    with tile.TileContext(nc) as tc:
        with (
            tc.tile_pool(name="xt", bufs=1) as xt_pool,
            tc.tile_pool(name="act", bufs=1) as act_pool,
            tc.tile_pool(name="w1", bufs=6) as w1_pool,
            tc.tile_pool(name="w2", bufs=3) as w2_pool,
            tc.tile_pool(name="tg", bufs=6) as tg_pool,
            tc.tile_pool(name="yo", bufs=3) as yo_pool,
            tc.tile_pool(name="warm", bufs=1) as warm_pool,
            tc.tile_pool(name="ps", bufs=8, space="PSUM") as ps_pool,
        ):
            # HAM warmup: a burst of throwaway matmuls on a zeroed scratch
            # tile keeps the PE busy from the end of the preamble, so the
            # clock gate is already at 8/8 when the first real matmul's
            # input lands (~4us of streaming activity flips it).
            warm_sb = warm_pool.tile([P, P], bf16)
            nc.vector.memset(warm_sb[:], 0)
            warm_ps = ps_pool.tile([P, 512], f32, tag="ps", name="warm_ps")
            for _ in range(44):
                nc.tensor.matmul(warm_ps[:, :P], warm_sb[:], warm_sb[:])
            # DMA plan. Both HWDGE rings carry the opening set in exact PE
            # consumption order: scalar takes xt rows k0-k10, sync takes the
            # j0 strips (halved so the first matmul starts ~1us in) + the
            # remaining xt rows, then the strip stream. In ffn2 the y output
            # stream alternates rings.
            w1_tiles = {}

            def issue_w1(m):
                t = w1_pool.tile([P, KO, P], bf16, tag="w1s")
                nc.sync.dma_start(t[:], w1_d.ap()[:, m])
                w1_tiles[m] = t

            xt_sb = xt_pool.tile([P, KO, C], bf16)

            # xt pieces grow geometrically: small head for a fast first
            # matmul, large tail to amortize the per-DMA completion cost.
            # The last piece rides the sync ring once j0's strips are in.
            XT_PIECES = [(0, 1), (1, 2), (2, 3), (3, 4), (4, 6), (6, 8), (8, 12)]
            for k0, k1 in XT_PIECES:
                nc.scalar.dma_start(xt_sb[:, k0:k1], xt_d.ap()[:, k0:k1])
            # j0's gate/up strips arrive as interleaved quarters because j0
            # interleaves its gate/up matmuls (see below).
            tg0 = w1_pool.tile([P, KO, P], bf16, tag="w1s", name="wg0")
            tu0 = w1_pool.tile([P, KO, P], bf16, tag="w1s", name="wu0")
            for k0, k1 in ((0, 4), (4, 8), (8, 16)):
                nc.sync.dma_start(tg0[:, k0:k1], w1_d.ap()[:, 0, k0:k1])
                nc.sync.dma_start(tu0[:, k0:k1], w1_d.ap()[:, 1, k0:k1])
            w1_tiles[0] = tg0
            w1_tiles[1] = tu0
            nc.sync.dma_start(xt_sb[:, 12:16], xt_d.ap()[:, 12:16])
            issue_w1(2)
            issue_w1(3)
            issue_w1(4)
            issue_w1(5)

            act_sb = act_pool.tile([P, MJ, C], bf16)

            with nc.named_scope("ffn1"):
                for j in range(MJ):
                    # prefetch strips three j-pairs ahead
                    if 2 * j + 6 < 2 * MJ:
                        issue_w1(2 * j + 6)
                    if 2 * j + 7 < 2 * MJ:
                        issue_w1(2 * j + 7)
                    wg = w1_tiles.pop(2 * j)
                    wu = w1_tiles.pop(2 * j + 1)
                    pg = [ps_pool.tile([P, 512], f32, tag="ps", name=f"pg{ci}") for ci in range(len(chunks))]
                    if j == 0:
                        # j0 runs while xt is still streaming in: interleave
                        # gate and up per k-step so each xt row feeds 6
                        # matmuls and PE demand matches the DMA supply rate.
                        pu = [ps_pool.tile([P, 512], f32, tag="ps", name=f"pu{ci}") for ci in range(len(chunks))]
                        for ko in range(KO):
                            for w_t, ps in ((wg, pg), (wu, pu)):
                                for ci, (c0, cn) in enumerate(chunks):
                                    nc.tensor.matmul(
                                        ps[ci][:, :cn],
                                        w_t[:, ko],
                                        xt_sb[:, ko, c0 : c0 + cn],
                                        start=(ko == 0),
                                        stop=(ko == KO - 1),
                                    )
                        tg = [tg_pool.tile([P, 512], f32, tag="tg", name=f"tg{ci}") for ci in range(len(chunks))]
                        for ci, (c0, cn) in enumerate(chunks):
                            nc.scalar.activation(tg[ci][:, :cn], pg[ci][:, :cn], act_fn)
                    else:
                        for ko in range(KO):
                            for ci, (c0, cn) in enumerate(chunks):
                                nc.tensor.matmul(
                                    pg[ci][:, :cn],
                                    wg[:, ko],
                                    xt_sb[:, ko, c0 : c0 + cn],
                                    start=(ko == 0),
                                    stop=(ko == KO - 1),
                                )
                        # gelu(gate) on ScalarE overlaps the up-projection matmuls
                        tg = [tg_pool.tile([P, 512], f32, tag="tg", name=f"tg{ci}") for ci in range(len(chunks))]
                        for ci, (c0, cn) in enumerate(chunks):
                            nc.scalar.activation(tg[ci][:, :cn], pg[ci][:, :cn], act_fn)
                        pu = [ps_pool.tile([P, 512], f32, tag="ps", name=f"pu{ci}") for ci in range(len(chunks))]
                        for ko in range(KO):
                            for ci, (c0, cn) in enumerate(chunks):
                                nc.tensor.matmul(
                                    pu[ci][:, :cn],
                                    wu[:, ko],
                                    xt_sb[:, ko, c0 : c0 + cn],
                                    start=(ko == 0),
                                    stop=(ko == KO - 1),
                                )
                    for ci, (c0, cn) in enumerate(chunks):
                        nc.vector.tensor_mul(
                            out=act_sb[:, j, c0 : c0 + cn],
                            in0=tg[ci][:, :cn],
                            in1=pu[ci][:, :cn],
                        )

            with nc.named_scope("ffn2"):
                w2_tiles = {}

                def issue_w2(i):
                    t = w2_pool.tile([P, MJ, P], bf16, tag="w2s")
                    nc.sync.dma_start(t[:], w2_d.ap()[:, i])
                    w2_tiles[i] = t

                issue_w2(0)
                issue_w2(1)
                for i in range(KO):
                    if i + 2 < KO:
                        issue_w2(i + 2)
                    w2t = w2_tiles.pop(i)
                    py = [ps_pool.tile([P, 512], f32, tag="ps", name=f"py{ci}") for ci in range(len(chunks))]
                    for fo in range(MJ):
                        for ci, (c0, cn) in enumerate(chunks):
                            nc.tensor.matmul(
                                py[ci][:, :cn],
                                w2t[:, fo],
                                act_sb[:, fo, c0 : c0 + cn],
                                start=(fo == 0),
                                stop=(fo == MJ - 1),
                            )
                    # evacuate all chunks into one wide SBUF tile (copies
                    # split across ScalarE+VectorE), then exactly one store
                    # per ring so the DMA completion receipts overlap
                    yo = yo_pool.tile([P, C], f32, tag="yo")
                    for ci, (c0, cn) in enumerate(chunks):
                        eng = nc.scalar.copy if ci == 1 else nc.vector.tensor_copy
                        eng(out=yo[:, c0 : c0 + cn], in_=py[ci][:, :cn])
                    h = (C // 2) & ~1
                    nc.scalar.dma_start(yt_d.ap()[:, i, :h], yo[:, :h])
                    nc.sync.dma_start(yt_d.ap()[:, i, h:], yo[:, h:])

    nc.compile()
    _BUILD_CACHE[key] = nc
    return nc


def _router(x, router_scale, gate_w):
    """Replicate the reference router ops exactly (same jax ops, default
    backend) so the top-2 expert selection bit-matches the reference."""
    import jax
    import jax.numpy as jnp

    x = jnp.asarray(x)
    router_scale = jnp.asarray(router_scale)
    gate_w = jnp.asarray(gate_w)
    _B, _L, d = x.shape
    h = x * jax.lax.rsqrt(jnp.mean(x * x, axis=-1, keepdims=True) + EPS)
    h = h * (d**-0.5) * router_scale
    logits = (h @ gate_w).astype(jnp.float32)
    probs = jax.nn.softmax(logits, axis=-1)
    w, idx = jax.lax.top_k(probs, TOP_K)
    w = w / jnp.clip(jnp.sum(w, axis=-1, keepdims=True), 1e-12)
    w = w.astype(x.dtype)
    return (
        np.asarray(idx).reshape(-1, TOP_K),
        np.asarray(w).reshape(-1, TOP_K).astype(np.float32),
    )


def _pack_w1(gate_up_e: np.ndarray) -> np.ndarray:
    """[D, 2F] -> [P, 2*MJ, KO, P] bf16, gate/up 128-col strips interleaved."""
    g = gate_up_e[:, :F].reshape(D, MJ, P)
    u = gate_up_e[:, F:].reshape(D, MJ, P)
    w1p = np.empty((D, 2 * MJ, P), np.float32)
    w1p[:, 0::2] = g
    w1p[:, 1::2] = u
    # [D, 2MJ, P] -> [KO, P, 2MJ, P] -> [P, 2MJ, KO, P]
    return np.ascontiguousarray(
        w1p.reshape(KO, P, 2 * MJ, P).transpose(1, 2, 0, 3)
    ).astype(BF16)


def _pack_w2(down_e: np.ndarray) -> np.ndarray:
    """[F, D] -> [P, KO, MJ, P] bf16 (w2[p,i,fo,q] = W2[fo*128+p, i*128+q])."""
    return np.ascontiguousarray(
        down_e.reshape(MJ, P, KO, P).transpose(1, 2, 0, 3)
    ).astype(BF16)


def run_moe(x, router_scale, gate_w, gate_up, down, per_expert_scale, trace=False):
    from concourse import bass_utils

    x = np.asarray(x, dtype=np.float32)
    router_scale = np.asarray(router_scale, dtype=np.float32)
    gate_w = np.asarray(gate_w, dtype=np.float32)
    gate_up = np.asarray(gate_up, dtype=np.float32)
    down = np.asarray(down, dtype=np.float32)
    per_expert_scale = np.asarray(per_expert_scale, dtype=np.float32)

    B, L, d = x.shape
    N = B * L
    assert d == D and gate_up.shape == (E, D, 2 * F) and down.shape == (E, F, D)

    idxf, wf = _router(x, router_scale, gate_w)

    pair_expert = idxf.reshape(-1)
    pair_token = np.repeat(np.arange(N), TOP_K)
    pair_w = wf.reshape(-1) * per_expert_scale[pair_expert]

    order = np.argsort(pair_expert, kind="stable")
    tok_o = pair_token[order]
    w_o = pair_w[order]
    counts = np.bincount(pair_expert, minlength=E)
    offs = np.zeros(E + 1, np.int64)
    offs[1:] = np.cumsum(counts)

    # SBUF budget caps per-launch capacity; extreme routing imbalance falls
    # back to multiple launches over row segments of each expert's list.
    CMAX = 1536
    nseg = max(1, -(-int(counts.max()) // CMAX))
    seg_cap = -(-int(counts.max()) // nseg)
    C = max(64, -(-seg_cap // 2) * 2)

    nc = _build(C)

    xf = x.reshape(N, D)
    w1_packed = [_pack_w1(gate_up[e]) for e in range(E)]
    w2_packed = [_pack_w2(down[e]) for e in range(E)]

    contrib = np.empty((len(tok_o), D), np.float32)
    res = None
    for s in range(nseg):
        in_maps = []
        ranges = []
        for e in range(E):
            lo = min(offs[e] + s * C, offs[e + 1])
            hi = min(lo + C, offs[e + 1])
            toks = tok_o[lo:hi]
            ranges.append((lo, hi))
            xg = np.zeros((C, D), np.float32)
            xg[: len(toks)] = xf[toks]
            xt = np.ascontiguousarray(
                xg.T.reshape(KO, P, C).transpose(1, 0, 2)
            ).astype(BF16)
            in_maps.append({"xt": xt, "w1": w1_packed[e], "w2": w2_packed[e]})

        res = bass_utils.run_bass_kernel_spmd(
            nc, in_maps, core_ids=list(range(E)), trace=trace and s == 0
        )
        for e in range(E):
            lo, hi = ranges[e]
            yt = np.asarray(res.results[e]["yt"])  # [P, KO, C]
            ytd = yt.transpose(1, 0, 2).reshape(D, C)  # [D, C]
            contrib[lo:hi] = ytd[:, : hi - lo].T

    contrib *= w_o[:, None]

    s = np.argsort(tok_o, kind="stable")
    tok_s = tok_o[s]
    out = np.zeros((N, D), np.float32)
    if len(tok_s) == 2 * N and np.array_equal(tok_s[0::2], tok_s[1::2]):
        cs = contrib[s]
        out[tok_s[0::2]] = cs[0::2] + cs[1::2]
    else:  # defensive fallback (duplicate experts per token can't happen)
        np.add.at(out, tok_o, contrib)
    return out.reshape(B, L, D), res


def kernel(x, router_scale, gate_w, gate_up, down, per_expert_scale):
    out, _ = run_moe(x, router_scale, gate_w, gate_up, down, per_expert_scale)
    return out
